# revision 6
# baseline (speedup 1.0000x reference)
"""CantorMoELayer Trainium2 kernel.

Strategy (data-parallel over batch, 1 batch row per NeuronCore, no collectives):

Host side (cheap, O(P log P) + weight folding):
  * tokens sorted by fingerprint -> every expert's band is a contiguous,
    128-aligned-padded token range (routing baked at trace time)
  * pentachoron mean collapses:  sum_v (V . dirs[e,v]) = V . dsum[e]
  * all per-token scalars (gate, fused, mask) commute past the linear maps:
        rec_e = m*fused*gctl * (xn_e @ (wv[e] @ wout[e]))
        q_e   = gctl * (xn_e . (w_e * wv[e] @ dsum[e]))
    so the whole expert collapses to two small folded matrices per expert:
        W1q[e] = diag(gamma_e) @ [w1[e] | wq[e]]   (64 x 17)
        WF[e]  = diag(gamma_e) @ wv[e] @ wout[e]   (64 x 64)
  * LN gamma/beta folded into the weights/biases.

Device side, per core (2048 sorted tokens x 1024 features, fp32):
  * x tiles [128, 16*1024] resident in SBUF, updated in place, streamed out
  * LN stats: DVE reduce (sum x) + ACT Square/accum (sum x^2)
  * per (expert, 128-token group): xn0 = (x-mu)*rsig  ->  PE transpose ->
    xnT [64,128] as the *stationary* matmul operand; tiny moving operands
    give token-major outputs [128tok, *], so every per-token scalar is a
    native per-partition [128,1] operand. No broadcasts, no rec transposes.
  * qsum accumulated token-major; fused = qsum * inv(5*den*|T|);
    out slice += rec0 * (fused*gctl*mask).
"""

import sys
import hashlib

if "/opt/trn_rl_repo" not in sys.path:
    sys.path.insert(0, "/opt/trn_rl_repo")

import numpy as np

import concourse.bacc as bacc
import concourse.mybir as mybir
import concourse.tile as tile
from concourse.bass_utils import run_bass_kernel_spmd

B, P, D = 8, 2048, 1024
E, S, DK = 16, 64, 128
H = S // 4
OV = 0.5
EPS_LN = 1e-5
NT = P // 128           # 16 token tiles per core
F32 = mybir.dt.float32

_CACHE = {}


def _sigmoid(x):
    return 1.0 / (1.0 + np.exp(-x))


def _host_prepare(fingerprints, ln_gamma, ln_beta, w1, b1, w2, b2, alpha,
                  wv, penta, betas, wout, pos_embed, temperature):
    """All weight folding + routing. Returns dict of host constants."""
    f64 = np.float64

    # fusion weights (weights only)
    dirs = penta / np.linalg.norm(penta, axis=-1, keepdims=True)
    dsum = dirs.sum(axis=1)                                   # [E, DK]
    pos_w = _sigmoid(pos_embed.mean(-1))                      # [E]
    offs = np.array([-2, -1, 1, 2])
    nbr = np.arange(E)[:, None] + offs[None, :]
    valid = ((nbr >= 0) & (nbr < E)).astype(np.float32)
    beta_w = 1.0 + (_sigmoid(betas) * valid).sum(-1) / valid.sum(-1)
    w_e = (pos_w * beta_w).astype(np.float32)                 # [E]
    aw = _sigmoid(alpha).astype(np.float32)                   # [E]

    wq = np.einsum('esd,ed->es', wv, dsum) * w_e[:, None]     # [E, S]
    wfold = np.einsum('esd,edt->est', wv, wout)               # [E, S, S]

    gam = ln_gamma.reshape(E, S)
    bet = ln_beta.reshape(E, S)
    w1q = np.concatenate([w1, wq[:, :, None]], axis=2)        # [E, S, H+1]
    w1q_f = gam[:, :, None] * w1q                             # gamma folded
    wfold_f = gam[:, :, None] * wfold
    brow = np.einsum('es,esh->eh', bet, w1q)                  # [E, H+1]
    brow[:, :H] += b1
    brec = np.einsum('es,est->et', bet, wfold)                # [E, S]

    # routing: sort by fingerprint; bands in f32-exact dyadic arithmetic
    e_ids = np.arange(E, dtype=np.float32)
    ext = np.float32(OV / E)
    fp_min = np.maximum(np.float32(0.0), e_ids / np.float32(E) - ext)
    fp_max = np.minimum(np.float32(1.0), (e_ids + 1) / np.float32(E) + ext)

    perm = np.argsort(fingerprints, kind='stable')
    fp_s = fingerprints[perm]
    starts = np.searchsorted(fp_s, fp_min, side='left')
    ends = np.searchsorted(fp_s, fp_max, side='left')
    pstarts = (starts // 128) * 128
    pends = np.minimum(((ends + 127) // 128) * 128, P)

    m_sorted = np.zeros((E, P), dtype=np.float32)
    for e in range(E):
        m_sorted[e, starts[e]:ends[e]] = 1.0
    den_s = np.maximum((m_sorted * w_e[:, None]).sum(0), 1e-6)
    inv5t = (1.0 / (5.0 * den_s * np.abs(f64(temperature)))).astype(np.float32)
    inv5t_col = inv5t.reshape(NT, 128).T.copy()               # [128, NT] token-major

    groups = []                                               # (expert, tile)
    for e in range(E):
        for i in range(pstarts[e] // 128, pends[e] // 128):
            groups.append((e, i))
    G = len(groups)

    ind = np.zeros((128, G), dtype=np.float32)                # in-band mask
    for g, (e, i) in enumerate(groups):
        ind[:, g] = m_sorted[e, i * 128:(i + 1) * 128]

    # const device tensors
    consts = {
        "identity": np.eye(128, dtype=np.float32),
        "w1q": np.ascontiguousarray(
            w1q_f.transpose(1, 0, 2).reshape(S, E * (H + 1))).astype(np.float32),
        "wfold": np.ascontiguousarray(
            wfold_f.transpose(1, 0, 2).reshape(S, E * S)).astype(np.float32),
        "w2b": np.broadcast_to(
            w2.reshape(1, E * H), (128, E * H)).astype(np.float32).copy(),
        "ind": ind,
        "inv5t": inv5t_col,
    }
    brow_nz = bool(np.any(brow != 0.0))
    brec_nz = bool(np.any(brec != 0.0))
    if brow_nz:
        consts["browb"] = np.broadcast_to(
            brow.reshape(1, E * (H + 1)), (128, E * (H + 1))).astype(np.float32).copy()
    if brec_nz:
        consts["brecb"] = np.broadcast_to(
            brec.reshape(1, E * S), (128, E * S)).astype(np.float32).copy()

    return {
        "perm": perm,
        "groups": groups,
        "consts": consts,
        "aw": aw,
        "b2": b2.astype(np.float32),
        "brow_nz": brow_nz,
        "brec_nz": brec_nz,
    }


def _build_module(hp):
    """Trace + compile the per-core SPMD program. Returns compiled nc."""
    groups = hp["groups"]
    G = len(groups)
    aw, b2 = hp["aw"], hp["b2"]
    brow_nz, brec_nz = hp["brow_nz"], hp["brec_nz"]

    nc = bacc.Bacc("TRN2", target_bir_lowering=False, debug=False, num_devices=8)

    # register activation-bias constants not in the builtin {0.0, 1.0} set
    need_consts = {float(-b) for b in b2} - {0.0, 1.0}
    for v in sorted(need_consts):
        t = nc.alloc_sbuf_tensor(f"const-float32-{v}", [128, 1], F32)
        nc.gpsimd.memset(t.ap(), v)
        nc.const_aps.aps[(F32, v)] = t.ap()
    if need_consts:
        nc.all_engine_barrier()

    xs = nc.dram_tensor("xs", [P, D], F32, kind="ExternalInput").ap()
    out = nc.dram_tensor("out", [P, D], F32, kind="ExternalOutput").ap()
    c_id = nc.dram_tensor("identity", [128, 128], F32, kind="ExternalInput").ap()
    c_w1q = nc.dram_tensor("w1q", [S, E * (H + 1)], F32, kind="ExternalInput").ap()
    c_wf = nc.dram_tensor("wfold", [S, E * S], F32, kind="ExternalInput").ap()
    c_w2b = nc.dram_tensor("w2b", [128, E * H], F32, kind="ExternalInput").ap()
    c_ind = nc.dram_tensor("ind", [128, G], F32, kind="ExternalInput").ap()
    c_i5t = nc.dram_tensor("inv5t", [128, NT], F32, kind="ExternalInput").ap()
    c_browb = c_brecb = None
    if brow_nz:
        c_browb = nc.dram_tensor("browb", [128, E * (H + 1)], F32,
                                 kind="ExternalInput").ap()
    if brec_nz:
        c_brecb = nc.dram_tensor("brecb", [128, E * S], F32,
                                 kind="ExternalInput").ap()

    xs_t = xs.rearrange("(i p) d -> p i d", p=128)            # [128, NT, D]
    out_t = out.rearrange("(i p) d -> p i d", p=128)

    with tile.TileContext(nc) as tc:
        with (
            tc.tile_pool(name="persist", bufs=1) as pp,
            tc.tile_pool(name="work", bufs=3) as wp,
            tc.tile_pool(name="scratch", bufs=2) as sp,
            tc.tile_pool(name="pt", bufs=2, space="PSUM") as ppt,
            tc.tile_pool(name="ph", bufs=2, space="PSUM") as pph,
            tc.tile_pool(name="pr", bufs=2, space="PSUM") as ppr,
        ):
            # ---- persistent SBUF ----
            x_sb = pp.tile([128, NT, D], F32)                 # 64KB/part
            xnt_all = pp.tile([S, G, 128], F32)               # staged xnT
            rec_all = pp.tile([128, G, S], F32)               # staged rec0
            gct_all = pp.tile([128, G], F32)
            qsum = pp.tile([128, NT], F32)
            stats = pp.tile([128, 8, NT], F32)  # 0:sum 1:sumsq 2:mu 3:rsig 4..7 scr
            ftok = pp.tile([128, NT], F32)

            id_sb = pp.tile([128, 128], F32)
            w1q_sb = pp.tile([S, E * (H + 1)], F32)
            wf_sb = pp.tile([S, E * S], F32)
            w2b_sb = pp.tile([128, E * H], F32)
            ind_sb = pp.tile([128, G], F32)
            i5t_sb = pp.tile([128, NT], F32)
            browb_sb = pp.tile([128, E * (H + 1)], F32) if brow_nz else None
            brecb_sb = pp.tile([128, E * S], F32) if brec_nz else None

            # ---- const loads ----
            nc.sync.dma_start(id_sb[:], c_id[:])
            nc.sync.dma_start(w1q_sb[:], c_w1q[:])
            nc.sync.dma_start(wf_sb[:], c_wf[:])
            nc.sync.dma_start(w2b_sb[:], c_w2b[:])
            nc.sync.dma_start(ind_sb[:], c_ind[:])
            nc.sync.dma_start(i5t_sb[:], c_i5t[:])
            if brow_nz:
                nc.sync.dma_start(browb_sb[:], c_browb[:])
            if brec_nz:
                nc.sync.dma_start(brecb_sb[:], c_brecb[:])

            # ---- x in (8 DMAs x 1MB) ----
            for g in range(8):
                nc.sync.dma_start(
                    x_sb[:, 2 * g:2 * g + 2, :], xs_t[:, 2 * g:2 * g + 2, :])

            nc.vector.memset(qsum[:], 0.0)

            # ---- LN stats ----
            for i in range(NT):
                xt = x_sb[:, i, :]
                nc.vector.tensor_reduce(
                    stats[:, 0, i:i + 1], xt, axis=mybir.AxisListType.X,
                    op=mybir.AluOpType.add)
                scr = sp.tile([128, D], F32, tag="sqscr")
                nc.scalar.activation(
                    scr[:], xt, mybir.ActivationFunctionType.Square,
                    accum_out=stats[:, 1, i:i + 1])
            # mu = sum/D ; v = var+eps = (sumsq/D + eps) - mu^2
            nc.vector.tensor_scalar_mul(stats[:, 2, :], stats[:, 0, :], 1.0 / D)
            nc.vector.tensor_scalar(
                stats[:, 4, :], stats[:, 1, :], 1.0 / D, float(EPS_LN),
                op0=mybir.AluOpType.mult, op1=mybir.AluOpType.add)
            nc.vector.tensor_mul(stats[:, 5, :], stats[:, 2, :], stats[:, 2, :])
            nc.vector.tensor_sub(stats[:, 6, :], stats[:, 4, :], stats[:, 5, :])
            # rsig = rsqrt(v) via Newton (no Sqrt on ACT: stay in one PWP set).
            # seed y0 = (1 + 1/v)/2 ~ rsqrt(v) near v=1 (LN variances ~1);
            # 4 Newton steps -> fp32 accuracy for v in [0.2, 4.5].
            v_ap = stats[:, 6, :]
            nc.vector.reciprocal(stats[:, 4, :], v_ap)
            nc.vector.tensor_scalar(
                stats[:, 3, :], stats[:, 4, :], 0.5, 0.5,
                op0=mybir.AluOpType.mult, op1=mybir.AluOpType.add)
            for _ in range(4):
                nc.vector.tensor_mul(stats[:, 5, :], stats[:, 3, :], stats[:, 3, :])
                nc.vector.tensor_mul(stats[:, 5, :], stats[:, 5, :], v_ap)
                nc.vector.tensor_scalar(
                    stats[:, 5, :], stats[:, 5, :], -0.5, 1.5,
                    op0=mybir.AluOpType.mult, op1=mybir.AluOpType.add)
                nc.vector.tensor_mul(stats[:, 3, :], stats[:, 3, :], stats[:, 5, :])

            # ---- pass A: per (expert, token-tile) group ----
            for g, (e, i) in enumerate(groups):
                mu_c = stats[:, 2, i:i + 1]
                rs_c = stats[:, 3, i:i + 1]
                xslice = x_sb[:, i, e * S:(e + 1) * S]        # [128, 64]

                xn0 = wp.tile([128, S], F32, tag="xn0")
                nc.vector.tensor_scalar(
                    xn0[:], xslice, mu_c, rs_c,
                    op0=mybir.AluOpType.subtract, op1=mybir.AluOpType.mult)

                pt = ppt.tile([S, 128], F32, tag="pt")        # xn0.T
                nc.tensor.transpose(pt[:], xn0[:], id_sb[:])
                nc.scalar.copy(xnt_all[:, g, :], pt[:])

                xnt_g = xnt_all[:, g, :]                      # [64, 128] lhsT
                ph = pph.tile([128, H + 1], F32, tag="ph")
                nc.tensor.matmul(ph[:], xnt_g, w1q_sb[:, e * (H + 1):(e + 1) * (H + 1)],
                                 start=True, stop=True)
                if brow_nz:
                    nc.vector.tensor_add(
                        ph[:], ph[:], browb_sb[:, e * (H + 1):(e + 1) * (H + 1)])

                pr = ppr.tile([128, S], F32, tag="pr")
                nc.tensor.matmul(pr[:], xnt_g, wf_sb[:, e * S:(e + 1) * S],
                                 start=True, stop=True)
                if brec_nz:
                    nc.vector.tensor_add(
                        pr[:], pr[:], brecb_sb[:, e * S:(e + 1) * S])
                nc.scalar.copy(rec_all[:, g, :], pr[:])

                # gelu(h) = h * (0.5*erf(h/sqrt(2)) + 0.5)   (erf-exact)
                hs = wp.tile([128, H], F32, tag="hs")
                eb = wp.tile([128, H], F32, tag="eb")
                nc.scalar.activation(eb[:], ph[:, 0:H],
                                     mybir.ActivationFunctionType.Erf,
                                     scale=0.7071067811865476)
                nc.vector.tensor_scalar(
                    eb[:], eb[:], 0.5, 0.5,
                    op0=mybir.AluOpType.mult, op1=mybir.AluOpType.add)
                nc.vector.tensor_mul(hs[:], eb[:], ph[:, 0:H])

                # z = h . w2[e] + b2 ; gct = 1 - aw*sigmoid(-z)
                zs = wp.tile([128, 2], F32, tag="zs")
                scr16 = wp.tile([128, H], F32, tag="scr16")
                nc.vector.tensor_mul(scr16[:], hs[:], w2b_sb[:, e * H:(e + 1) * H])
                nc.vector.tensor_reduce(
                    zs[:, 0:1], scr16[:], axis=mybir.AxisListType.X,
                    op=mybir.AluOpType.add)
                nc.scalar.activation(zs[:, 1:2], zs[:, 0:1],
                                     mybir.ActivationFunctionType.Sigmoid,
                                     scale=-1.0, bias=-float(b2[e]))
                nc.vector.tensor_scalar(
                    gct_all[:, g:g + 1], zs[:, 1:2], -float(aw[e]), 1.0,
                    op0=mybir.AluOpType.mult, op1=mybir.AluOpType.add)

                # qsum[:, i] += q * gct * ind
                qt = wp.tile([128, 2], F32, tag="qt")
                nc.vector.tensor_mul(qt[:, 0:1], ph[:, H:H + 1], gct_all[:, g:g + 1])
                nc.vector.tensor_mul(qt[:, 1:2], qt[:, 0:1], ind_sb[:, g:g + 1])
                nc.vector.tensor_add(qsum[:, i:i + 1], qsum[:, i:i + 1], qt[:, 1:2])

            # ---- fused ----
            nc.vector.tensor_mul(ftok[:], qsum[:], i5t_sb[:])

            # ---- pass B: out slice += rec0 * (fused*gct*ind) ----
            for g, (e, i) in enumerate(groups):
                gcol = wp.tile([128, 2], F32, tag="gcol")
                nc.vector.tensor_mul(gcol[:, 0:1], ftok[:, i:i + 1],
                                     gct_all[:, g:g + 1])
                nc.vector.tensor_mul(gcol[:, 1:2], gcol[:, 0:1], ind_sb[:, g:g + 1])
                recs = wp.tile([128, S], F32, tag="recs")
                nc.scalar.activation(recs[:], rec_all[:, g, :],
                                     mybir.ActivationFunctionType.Copy,
                                     scale=gcol[:, 1:2])
                xslice = x_sb[:, i, e * S:(e + 1) * S]
                nc.vector.tensor_add(xslice, xslice, recs[:])

            # ---- out ----
            for g in range(8):
                nc.sync.dma_start(
                    out_t[:, 2 * g:2 * g + 2, :], x_sb[:, 2 * g:2 * g + 2, :])

    nc.compile()
    return nc


def _get_compiled(inputs):
    key = hashlib.sha256(b"".join(
        np.ascontiguousarray(inputs[k]).tobytes()
        for k in sorted(inputs) if k != "x")).hexdigest()
    if key not in _CACHE:
        hp = _host_prepare(**{k: v for k, v in inputs.items() if k != "x"})
        nc = _build_module(hp)
        _CACHE[key] = (nc, hp)
    return _CACHE[key]


def kernel(**inputs):
    inputs = {k: np.asarray(v) for k, v in inputs.items()}
    nc, hp = _get_compiled(inputs)
    perm = hp["perm"]
    consts = hp["consts"]

    x = inputs["x"].astype(np.float32, copy=False)
    x_s = np.ascontiguousarray(x[:, perm, :])                 # sorted tokens

    in_maps = []
    for b in range(B):
        m = {"xs": x_s[b]}
        m.update(consts)
        in_maps.append(m)

    res = run_bass_kernel_spmd(nc, in_maps, core_ids=list(range(B)))
    out = np.empty((B, P, D), dtype=np.float32)
    inv = np.empty_like(perm)
    inv[perm] = np.arange(P)
    for b in range(B):
        out[b] = res.results[b]["out"][inv]
    return out
